# revision 97
# baseline (speedup 1.0000x reference)
"""AmplitudeQuantumNet Trainium2 kernel (8-core data parallel).

Pipeline per core (128 samples), 8 chunks of 16 samples, conv2 emitted
LAG chunks behind conv1 so the PE never idles (keeps the HAM clock gate
at 2.4 GHz -- idle gaps re-throttle it to 1.2 GHz):
  conv1(1->32,3x3)+BN+bias -> relu+pool-x        [K=19 im2col matmul;
                                                  ACT odd-col extract + STT]
  pool-y + x-shift replication                   [vector max pair + contiguous
                                                  byte-shifted SBUF DMAs]
  conv2(32->64,3x3)+BN (parity-split dy)         [6 K=96 matmuls/subtile,
                                                  392-col free dim]
  4-way max (y-parity x pool-x)                  [one vector reduce, axis=XY]
  relu+bias scatter to fc layout                 [scalar ACT, (ypm,xp,s)
                                                  iteration: contiguous runs]
  fc(3136->256)+tanh                             [K=128 (c + 64*(yp//4)),
                                                  2x28 matmuls, contiguous rhs]
  quantum statevector sim                        [host-built 256x256 unitary,
                                                  real+imag bf16 matmuls]
  probs -> Z expvals (norm via unitarity) -> MLP [tiny matmuls]

conv2 parity trick: conv1-pooled rows are kept split by y-parity (E=even
pooled rows, O=odd).  conv2 output rows 2yy / 2yy+1 are computed as two
separate 3-matmul accumulations over (E, O) row windows, and maxpool-y
then reduces the two parities at equal yy -- no interleave copies needed.
x-shifts (+-1 col) for the 3x3 kernel are physical partition-block copies
made with single contiguous byte-shifted SBUF DMAs (pad cols absorb the
wrap).

DMA notes (this axon/VNC setup): each DMA completion semaphore fires
~4us after the transfer visibly ends, and triggers cost ~0.6-1us on the
issuing engine's queue, so: xim chunk loads are all pre-issued on the
sync ring ahead of the per-chunk shift DMAs, the big tail weights ride
the same ring right after them, and everything latency-critical intra-
chunk (pool-y max, rc copy) stays on compute engines.
"""

import sys

sys.path.insert(0, "/opt/trn_rl_repo")

import numpy as np
import ml_dtypes

BF16 = ml_dtypes.bfloat16

N_QUBITS = 8
Q_DEPTH = 10
DIM = 256
BN_EPS = 1e-5
B = 1024
NCORES = 8
B_CORE = B // NCORES  # 128
SUB = 4               # samples per pipeline subtile
NCH = 8               # chunks per core
CH = B_CORE // NCH    # 16 samples per chunk
SPC = CH // SUB       # 4 subtiles per chunk
JY = [0, 2, 1, 3]     # conv1 partition-block -> jy shift (pool pairs at +-64)

_CACHE = {}


# ---------------------------------------------------------------- host precompute
def _quantum_unitary(q_params):
    """256x256 complex matrix of the full circuit (H layer + 10x[RX layer + diag])."""
    bits = ((np.arange(DIM)[:, None] >> (N_QUBITS - 1 - np.arange(N_QUBITS))) & 1)
    ph = np.where(np.arange(N_QUBITS) % 2 == 0, 1j, np.exp(1j * np.pi / 4))
    diag = np.prod(np.power(ph[None, :], bits), axis=1)
    cz = np.ones(DIM)
    for i, j in [(0, 1), (2, 3), (4, 5), (6, 7), (1, 2), (3, 4), (5, 6)]:
        cz = cz * ((-1.0) ** (bits[:, i] * bits[:, j]))
    diagc = (diag * cz).astype(np.complex128)

    def app(M, U, w):
        M = M.reshape((2,) * N_QUBITS + (DIM,))
        M = np.moveaxis(M, w, 0)
        M = np.tensordot(U, M, axes=(1, 0))
        M = np.moveaxis(M, 0, w)
        return M.reshape(DIM, DIM)

    M = np.eye(DIM, dtype=np.complex128)
    H = np.array([[1.0, 1.0], [1.0, -1.0]]) / np.sqrt(2.0)
    for w in range(N_QUBITS):
        M = app(M, H, w)
    qw = np.asarray(q_params, np.float64).reshape(Q_DEPTH, N_QUBITS)
    X = np.array([[0.0, 1.0], [1.0, 0.0]])
    I2 = np.eye(2)
    for layer in range(Q_DEPTH):
        for w in range(N_QUBITS):
            t = qw[layer, w]
            U = np.cos(t / 2) * I2 - 1j * np.sin(t / 2) * X
            M = app(M, U, w)
        M = diagc[:, None] * M
    zsigns = (1 - 2 * bits).astype(np.float64)  # [256, 8]
    return M, zsigns


def _host_prep(inputs):
    f32 = np.float32
    x = np.asarray(inputs["x"], f32)  # [1024,1,28,28]

    inv1 = inputs["bn1_gamma"] / np.sqrt(inputs["bn1_var"] + BN_EPS)
    w1f = np.asarray(inputs["conv1_w"], f32) * inv1[:, None, None, None]
    b1f = (inputs["conv1_b"] - inputs["bn1_mean"]) * inv1 + inputs["bn1_beta"]
    inv2 = inputs["bn2_gamma"] / np.sqrt(inputs["bn2_var"] + BN_EPS)
    w2f = np.asarray(inputs["conv2_w"], f32) * inv2[:, None, None, None]
    b2f = (inputs["conv2_b"] - inputs["bn2_mean"]) * inv2 + inputs["bn2_beta"]

    # conv1 lhsT [19, 128]: rows (r6,dxc) + bias row; cols (jyblk, c)
    W1 = np.zeros((19, 128), f32)
    for blk in range(4):
        jy = JY[blk]
        for r6 in range(6):
            dy = r6 - jy
            if 0 <= dy <= 2:
                for dxc in range(3):
                    W1[r6 * 3 + dxc, blk * 32:(blk + 1) * 32] = w1f[:, 0, dy, dxc]
        W1[18, blk * 32:(blk + 1) * 32] = b1f
    W1 = np.ascontiguousarray(W1, f32).astype(BF16)

    # x im2col [1024, 19, 7, 28]: row p=(r6,dxc): xpad[s, 4q+r6, x+dxc]; row 18 = 1
    xp = np.zeros((B, 30, 30), f32)
    xp[:, 1:29, 1:29] = x[:, 0]
    xim = np.empty((B, 19, 7, 28), f32)
    for r6 in range(6):
        for dxc in range(3):
            xim[:, r6 * 3 + dxc] = xp[:, r6:r6 + 25:4, dxc:dxc + 28]
    xim[:, 18] = 1.0
    # per-core partition-major [19, 128, 7, 28]
    xim_cores = [
        np.ascontiguousarray(xim[i * B_CORE:(i + 1) * B_CORE].transpose(1, 0, 2, 3)).astype(BF16)
        for i in range(NCORES)
    ]

    # conv2 lhsT pair, parity-split.  E-tile blocks: [center(k=1), +1(k=0), -1(k=2)];
    # O-tile blocks: [+1(k=0), center(k=1), -1(k=2)].
    wt = w2f.transpose(1, 2, 0, 3)  # [c, dy, m, k]
    W2E = np.zeros((96, 3, 64), f32)
    W2O = np.zeros((96, 3, 64), f32)
    for blk, (ke, ko) in enumerate(zip([1, 0, 2], [0, 1, 2])):
        W2E[blk * 32:(blk + 1) * 32] = wt[:, :, :, ke]
        W2O[blk * 32:(blk + 1) * 32] = wt[:, :, :, ko]
    W2E = W2E.astype(BF16)
    W2O = W2O.astype(BF16)

    # fc lhsT [128, 28, 2, 128]: K-row = c + 64*g (g = yp//4), j = (yp%4... g*4+ypm)*7+xp
    # p2full[c + 64g, ypm*7+xp, s] = pooled[c, 4g+ypm, xp, s]
    v = np.asarray(inputs["fc_w"], f32).reshape(2, 128, 64, 7, 7)  # [mt, m, c, yp, xp]
    fcw = np.zeros((128, 28, 2, 128), f32)
    fcw[0:64, 0:28] = v[:, :, :, 0:4, :].transpose(2, 3, 4, 0, 1).reshape(64, 28, 2, 128)
    fcw[64:128, 0:21] = v[:, :, :, 4:7, :].transpose(2, 3, 4, 0, 1).reshape(64, 21, 2, 128)
    fcw = np.ascontiguousarray(fcw).astype(BF16)
    fcb = np.ascontiguousarray(np.asarray(inputs["fc_b"], f32).reshape(2, 128).T)  # [128, 2]

    M, zsigns = _quantum_unitary(np.asarray(inputs["q_params"], np.float64))
    # lhsT tiles [k128, kb2, mt2, m128]: value M[mt*128+m, kb*128+k]
    mrt = M.real.T.reshape(2, 128, 2, 128).transpose(1, 0, 2, 3)
    mit = M.imag.T.reshape(2, 128, 2, 128).transpose(1, 0, 2, 3)
    mrt = np.ascontiguousarray(mrt).astype(f32).astype(BF16)
    mit = np.ascontiguousarray(mit).astype(f32).astype(BF16)
    zext = np.ones((DIM, 9), np.float64)
    zext[:, :8] = zsigns
    zext = np.ascontiguousarray(zext.reshape(2, 128, 9).transpose(1, 0, 2)).astype(f32).astype(BF16)

    p1t = np.ascontiguousarray(np.asarray(inputs["p1_w"], f32).T).astype(BF16)  # [8,128]
    p2t = np.ascontiguousarray(np.asarray(inputs["p2_w"], f32).T).astype(BF16)  # [128,64]
    p3t = np.ascontiguousarray(np.asarray(inputs["p3_w"], f32).T).astype(BF16)  # [64,10]

    common = {
        "w1": W1, "w2e": W2E, "w2o": W2O, "fcw": fcw, "fcb": fcb,
        "mrt": mrt, "mit": mit, "zext": zext,
        "p1t": p1t, "p2t": p2t, "p3t": p3t,
        "cb2": np.asarray(b2f, f32).reshape(64, 1),
        "pb1": np.asarray(inputs["p1_b"], f32).reshape(128, 1),
        "pb2": np.asarray(inputs["p2_b"], f32).reshape(64, 1),
        "pb3": np.asarray(inputs["p3_b"], f32).reshape(10, 1),
    }
    in_maps = []
    for i in range(NCORES):
        m = dict(common)
        m["xim"] = xim_cores[i]
        in_maps.append(m)
    return in_maps


# ---------------------------------------------------------------- bass program
def _build_bass():
    import concourse.bacc as bacc
    import concourse.mybir as mybir
    import concourse.tile as tile

    dt = mybir.dt
    AF = mybir.ActivationFunctionType
    ALU = mybir.AluOpType
    AX = mybir.AxisListType

    nc = bacc.Bacc("TRN2", target_bir_lowering=False, debug=False,
                   num_devices=NCORES)
    xim = nc.dram_tensor("xim", [19, B_CORE, 7, 28], dt.bfloat16, kind="ExternalInput")
    w1 = nc.dram_tensor("w1", [19, 128], dt.bfloat16, kind="ExternalInput")
    w2e = nc.dram_tensor("w2e", [96, 3, 64], dt.bfloat16, kind="ExternalInput")
    w2o = nc.dram_tensor("w2o", [96, 3, 64], dt.bfloat16, kind="ExternalInput")
    fcw = nc.dram_tensor("fcw", [128, 28, 2, 128], dt.bfloat16, kind="ExternalInput")
    fcb = nc.dram_tensor("fcb", [128, 2], dt.float32, kind="ExternalInput")
    mrt = nc.dram_tensor("mrt", [128, 2, 2, 128], dt.bfloat16, kind="ExternalInput")
    mit = nc.dram_tensor("mit", [128, 2, 2, 128], dt.bfloat16, kind="ExternalInput")
    zext = nc.dram_tensor("zext", [128, 2, 9], dt.bfloat16, kind="ExternalInput")
    p1t = nc.dram_tensor("p1t", [8, 128], dt.bfloat16, kind="ExternalInput")
    p2t = nc.dram_tensor("p2t", [128, 64], dt.bfloat16, kind="ExternalInput")
    p3t = nc.dram_tensor("p3t", [64, 10], dt.bfloat16, kind="ExternalInput")
    cb2 = nc.dram_tensor("cb2", [64, 1], dt.float32, kind="ExternalInput")
    pb1 = nc.dram_tensor("pb1", [128, 1], dt.float32, kind="ExternalInput")
    pb2 = nc.dram_tensor("pb2", [64, 1], dt.float32, kind="ExternalInput")
    pb3 = nc.dram_tensor("pb3", [10, 1], dt.float32, kind="ExternalInput")
    out = nc.dram_tensor("out", [10, B_CORE], dt.float32, kind="ExternalOutput")
    debug = bool(_CACHE.get("debug"))
    if debug:
        dbg_et = nc.dram_tensor("dbg_et", [96, CH, 8, 16], dt.bfloat16, kind="ExternalOutput")
        dbg_ot = nc.dram_tensor("dbg_ot", [96, CH, 8, 16], dt.bfloat16, kind="ExternalOutput")
        dbg_p2f = nc.dram_tensor("dbg_p2f", [128, 28, B_CORE], dt.bfloat16, kind="ExternalOutput")
        dbg_feats = nc.dram_tensor("dbg_feats", [128, 2, 128], dt.bfloat16, kind="ExternalOutput")
        dbg_p1c = nc.dram_tensor("dbg_p1c", [128, CH, 7, 14], dt.bfloat16, kind="ExternalOutput")
        dbg_t3 = nc.dram_tensor("dbg_t3", [64, SUB, 7, 7], dt.bfloat16, kind="ExternalOutput")
        dbg_pe = nc.dram_tensor("dbg_pe", [64, 2, 448], dt.float32, kind="ExternalOutput")

    NEO = CH * 8 * 16  # flat elements per partition of an E/O tile

    with tile.TileContext(nc) as tc:
        with tc.tile_pool(name="singles", bufs=1) as singles:
            # conv1-critical loads on the sync HWDGE ring, first.
            w1_sb = singles.tile([19, 128], dt.bfloat16, tag="w1")
            nc.sync.dma_start(out=w1_sb, in_=w1[:, :], single_packet=True)
            # everything small on the scalar HWDGE ring.
            w2e_sb = singles.tile([96, 3, 64], dt.bfloat16, tag="w2e")
            nc.scalar.dma_start(out=w2e_sb, in_=w2e[:, :, :])
            w2o_sb = singles.tile([96, 3, 64], dt.bfloat16, tag="w2o")
            nc.scalar.dma_start(out=w2o_sb, in_=w2o[:, :, :])
            cb2_sb = singles.tile([64, 1], dt.float32, tag="cb2")
            nc.scalar.dma_start(out=cb2_sb, in_=cb2[:, :])
            fcb_sb = singles.tile([128, 2], dt.float32, tag="fcb")
            nc.scalar.dma_start(out=fcb_sb, in_=fcb[:, :])
            p1t_sb = singles.tile([8, 128], dt.bfloat16, tag="p1t")
            nc.scalar.dma_start(out=p1t_sb, in_=p1t[:, :])
            p2t_sb = singles.tile([128, 64], dt.bfloat16, tag="p2t")
            nc.scalar.dma_start(out=p2t_sb, in_=p2t[:, :])
            p3t_sb = singles.tile([64, 10], dt.bfloat16, tag="p3t")
            nc.scalar.dma_start(out=p3t_sb, in_=p3t[:, :])
            pb1_sb = singles.tile([128, 1], dt.float32, tag="pb1")
            nc.scalar.dma_start(out=pb1_sb, in_=pb1[:, :])
            pb2_sb = singles.tile([64, 1], dt.float32, tag="pb2")
            nc.scalar.dma_start(out=pb2_sb, in_=pb2[:, :])
            pb3_sb = singles.tile([10, 1], dt.float32, tag="pb3")
            nc.scalar.dma_start(out=pb3_sb, in_=pb3[:, :])

            # big tail weights: tiles now, DMAs emitted mid-conv-loop (sync ring)
            fcw_sb = singles.tile([128, 28, 2, 128], dt.bfloat16, tag="fcw")
            mrt_sb = singles.tile([128, 2, 2, 128], dt.bfloat16, tag="mrt")
            mit_sb = singles.tile([128, 2, 2, 128], dt.bfloat16, tag="mit")
            zext_sb = singles.tile([128, 2, 9], dt.bfloat16, tag="zext")

            # fc input [c + 64*(yp//4), (yp%4)*7+xp, s] -- sample-innermost so
            # the fc matmul rhs is contiguous (strided rhs slows the PE ~4x)
            p2full = singles.tile([128, 28, B_CORE], dt.bfloat16, tag="p2full")
            # j=21..27 of the upper half is never written; fc reads it with
            # zero weights, so it must at least be finite.
            nc.gpsimd.memset(p2full[64:128, 21:28, :], 0.0)

            ones18 = singles.tile([1, 8], dt.bfloat16, tag="ones18")
            nc.gpsimd.memset(ones18, 1.0)

            # per-chunk conv2 input tiles (persistent; only pads need zeroing,
            # emitted per-chunk inside the loop to keep engine queues clear)
            Et = [singles.tile([96, CH, 8, 16], dt.bfloat16, tag=f"Et{ci}",
                               name=f"Et{ci}") for ci in range(NCH)]
            Ot = [singles.tile([96, CH, 8, 16], dt.bfloat16, tag=f"Ot{ci}",
                               name=f"Ot{ci}") for ci in range(NCH)]

            with tc.tile_pool(name="ximp", bufs=8) as ximpool, \
                 tc.tile_pool(name="oddp", bufs=6) as oddpool, \
                 tc.tile_pool(name="p1cp", bufs=4) as p1cpool, \
                 tc.tile_pool(name="rcp", bufs=4) as rcpool, \
                 tc.tile_pool(name="t3p", bufs=6) as t3pool, \
                 tc.tile_pool(name="ps1", bufs=2, space="PSUM") as psum1, \
                 tc.tile_pool(name="ps2", bufs=2, space="PSUM") as psum2:
                p1cs = {}

                def emit_pads(ci):
                    # zero only the pad regions of Et/Ot (rest is overwritten)
                    ef = Et[ci].rearrange("p a b c -> p (a b c)")
                    of = Ot[ci].rearrange("p a b c -> p (a b c)")
                    nc.gpsimd.memset(Et[ci][0:32, :, :, 14:16], 0.0)
                    nc.gpsimd.memset(Et[ci][0:32, :, 7:8, 0:14], 0.0)
                    nc.gpsimd.memset(ef[32:64, 0:1], 0.0)
                    nc.gpsimd.memset(ef[64:96, NEO - 1:NEO], 0.0)
                    nc.gpsimd.memset(ef[64:96, NEO // 2 - 1:NEO // 2], 0.0)
                    nc.gpsimd.memset(Ot[ci][32:64, :, :, 14:16], 0.0)
                    nc.gpsimd.memset(Ot[ci][32:64, :, 0:1, 0:14], 0.0)
                    nc.gpsimd.memset(of[0:32, 0:1], 0.0)
                    nc.gpsimd.memset(of[64:96, NEO - 1:NEO], 0.0)
                    nc.gpsimd.memset(of[64:96, NEO // 2 - 1:NEO // 2], 0.0)

                def emit_conv1(ci, tt0, tt1):
                    xim_sb = xim_sbs[ci]
                    if tt0 == 0:
                        p1c = p1cpool.tile([128, CH, 7, 14], dt.bfloat16, tag="p1c")
                        p1cs[ci] = p1c
                    else:
                        p1c = p1cs[ci]
                    for tt in range(tt0, tt1):
                        c1p = psum1.tile([128, SUB // 2, 512], dt.float32, tag="c1p")
                        for sh in range(SUB // 2):
                            s0 = tt * SUB + sh * 2
                            if ci == 0 and tt < 2:
                                xsrc = xim0a[:, s0:s0 + 2]
                            else:
                                xsrc = xim_sb[:, s0:s0 + 2]
                            nc.tensor.matmul(
                                c1p[:, sh, 0:392].rearrange(
                                    "p (s q x) -> p s q x", s=2, q=7, x=28),
                                w1_sb, xsrc, start=True, stop=True)
                        c1v = c1p[:, :, 0:392].rearrange(
                            "p h (s q xp two) -> p h s q xp two", s=2, q=7, xp=14, two=2)
                        odd1 = oddpool.tile([128, SUB, 7, 14], dt.bfloat16, tag="odd1")
                        o1v = odd1.rearrange("p (h s) q xp -> p h s q xp", h=SUB // 2)
                        nc.scalar.activation(o1v, c1v[:, :, :, :, :, 1], AF.Copy)
                        nc.vector.scalar_tensor_tensor(
                            p1c[:, tt * SUB:(tt + 1) * SUB].rearrange(
                                "p (h s) q xp -> p h s q xp", h=SUB // 2),
                            c1v[:, :, :, :, :, 0], 0.0, o1v, ALU.max, ALU.max)

                def emit_pooly(ci):
                    p1c = p1cs[ci]
                    rc = rcpool.tile([64, CH, 7, 14], dt.bfloat16, tag="rc")
                    nc.vector.tensor_copy(out=rc, in_=p1c[64:128])
                    # E rows 0..6 = even pooled rows; O rows 1..7 = odd pooled rows
                    nc.vector.tensor_tensor(
                        Et[ci][0:32, :, 0:7, 0:14], p1c[0:32], rc[0:32], ALU.max)
                    nc.vector.tensor_tensor(
                        Ot[ci][32:64, :, 1:8, 0:14], p1c[32:64], rc[32:64], ALU.max)
                    ef = Et[ci].rearrange("p a b c -> p (a b c)")
                    of = Ot[ci].rearrange("p a b c -> p (a b c)")
                    # +-1 column shifts as single contiguous byte-shifted copies;
                    # sync ring (xims are pre-issued, so nothing queues behind these)
                    nc.sync.dma_start(out=ef[32:64, 1:NEO], in_=ef[0:32, 0:NEO - 1])
                    nc.sync.dma_start(out=ef[64:96, 0:NEO - 1], in_=ef[0:32, 1:NEO])
                    nc.sync.dma_start(out=of[0:32, 1:NEO], in_=of[32:64, 0:NEO - 1])
                    nc.sync.dma_start(out=of[64:96, 0:NEO - 1], in_=of[32:64, 1:NEO])

                def emit_conv2(ci):
                    for tt in range(SPC):
                        s0 = tt * SUB
                        ps = psum2.tile([64, 2, 512], dt.float32, tag="c2p")
                        pe = ps[:, 0, 0:448].rearrange(
                            "p (s y x) -> p s y x", s=SUB, y=7, x=16)[:, :, :, 0:14]
                        po = ps[:, 1, 0:448].rearrange(
                            "p (s y x) -> p s y x", s=SUB, y=7, x=16)[:, :, :, 0:14]
                        Ev = Et[ci][:, s0:s0 + SUB]
                        Ov = Ot[ci][:, s0:s0 + SUB]
                        # even out rows: W[0]*O[yy-1] + W[1]*E[yy] + W[2]*O[yy]
                        nc.tensor.matmul(pe, w2o_sb[:, 0, :], Ov[:, :, 0:7, 0:14],
                                         start=True, stop=False)
                        nc.tensor.matmul(pe, w2e_sb[:, 1, :], Ev[:, :, 0:7, 0:14],
                                         start=False, stop=False)
                        nc.tensor.matmul(pe, w2o_sb[:, 2, :], Ov[:, :, 1:8, 0:14],
                                         start=False, stop=True)
                        # odd out rows: W[0]*E[yy] + W[1]*O[yy] + W[2]*E[yy+1]
                        nc.tensor.matmul(po, w2e_sb[:, 0, :], Ev[:, :, 0:7, 0:14],
                                         start=True, stop=False)
                        nc.tensor.matmul(po, w2o_sb[:, 1, :], Ov[:, :, 1:8, 0:14],
                                         start=False, stop=False)
                        nc.tensor.matmul(po, w2e_sb[:, 2, :], Ev[:, :, 1:8, 0:14],
                                         start=False, stop=True)
                        # fused maxpool 2x2: max over (y-parity, x-pair); junk
                        # col pair (14,15) excluded from the input AP
                        t3 = t3pool.tile([64, SUB, 7, 7], dt.bfloat16, tag="t3")
                        rin = ps[:, :, 0:448].rearrange(
                            "p par (s y xh two) -> p s y xh par two",
                            s=SUB, y=7, xh=8, two=2)[:, :, :, 0:7]
                        nc.vector.tensor_reduce(t3, rin, axis=AX.XY, op=ALU.max)
                        if debug and ci == 0 and tt == 0:
                            nc.sync.dma_start(out=dbg_t3[:, :, :, :], in_=t3)
                            pecp = t3pool.tile([64, 2, 448], dt.float32, tag="pecp")
                            nc.scalar.activation(pecp, ps[:, :, 0:448], AF.Copy)
                            nc.sync.dma_start(out=dbg_pe[:, :, :], in_=pecp)
                        # relu + conv2 bias, scatter to fc layout
                        gs = ci * CH + s0
                        # iterate (ypm, xp, s) so dst writes are contiguous
                        # 4-sample runs instead of stride-128 singles
                        dst0 = p2full[0:64].rearrange(
                            "p (ypm xp) s -> p ypm xp s", ypm=4)[:, :, :, gs:gs + SUB]
                        dst1 = p2full[64:128, 0:21, :].rearrange(
                            "p (ypm xp) s -> p ypm xp s", ypm=3)[:, :, :, gs:gs + SUB]
                        if ci == NCH - 1 and tt == SPC - 1:
                            # final subtile: emit on vector right after its own
                            # reduce so fc isn't gated on a scalar-queue hop
                            nc.vector.tensor_scalar(
                                dst0, t3[:, :, 0:4, :].rearrange("p s y x -> p y x s"),
                                cb2_sb[:, 0:1], 0.0, ALU.add, ALU.max)
                            nc.vector.tensor_scalar(
                                dst1, t3[:, :, 4:7, :].rearrange("p s y x -> p y x s"),
                                cb2_sb[:, 0:1], 0.0, ALU.add, ALU.max)
                        else:
                            nc.scalar.activation(
                                dst0, t3[:, :, 0:4, :].rearrange("p s y x -> p y x s"),
                                AF.Relu, bias=cb2_sb[:, 0:1])
                            nc.scalar.activation(
                                dst1, t3[:, :, 4:7, :].rearrange("p s y x -> p y x s"),
                                AF.Relu, bias=cb2_sb[:, 0:1])

                LAG = 4
                xim0a = ximpool.tile([19, 2 * SUB, 7, 28], dt.bfloat16, tag="xim0a")
                nc.sync.dma_start(out=xim0a, in_=xim[:, 0:2 * SUB, :, :],
                                  single_packet=True)
                xim_sbs = {}
                for ci in range(NCH):
                    xim_sbs[ci] = ximpool.tile([19, CH, 7, 28], dt.bfloat16,
                                               tag="xim_sb", name=f"xim{ci}")
                    nc.sync.dma_start(out=xim_sbs[ci],
                                      in_=xim[:, ci * CH:(ci + 1) * CH, :, :])
                nc.sync.dma_start(out=fcw_sb, in_=fcw[:, :, :, :])
                nc.sync.dma_start(out=mrt_sb, in_=mrt[:, :, :, :])
                nc.sync.dma_start(out=mit_sb, in_=mit[:, :, :, :])
                nc.sync.dma_start(out=zext_sb, in_=zext[:, :, :])
                emit_pads(0)
                emit_pads(1)
                for ci in range(NCH):
                    emit_conv1(ci, 0, SPC)
                    if ci + 2 < NCH:
                        emit_pads(ci + 2)
                    emit_pooly(ci)
                    if ci >= LAG:
                        emit_conv2(ci - LAG)
                for ci in range(NCH - LAG, NCH):
                    emit_conv2(ci)
                if debug:
                    nc.sync.dma_start(out=dbg_et[:, :, :, :], in_=Et[0])
                    nc.sync.dma_start(out=dbg_ot[:, :, :, :], in_=Ot[0])

            # ---------------- dense tail ----------------
            with tc.tile_pool(name="tail", bufs=1) as tail, \
                 tc.tile_pool(name="psumT", bufs=1, space="PSUM") as psumT:
                fp = psumT.tile([128, 2, 128], dt.float32, tag="fp")
                feats = tail.tile([128, 2, 128], dt.bfloat16, tag="feats")
                for mt in range(2):
                    for j in range(28):
                        nc.tensor.matmul(
                            fp[:, mt], fcw_sb[:, j, mt, :], p2full[:, j, :],
                            start=(j == 0), stop=(j == 27))
                    # tanh(mt) overlaps the next mt's matmuls on scalar
                    nc.scalar.activation(feats[:, mt], fp[:, mt], AF.Tanh,
                                         bias=fcb_sb[:, mt:mt + 1])

                sq = psumT.tile([128, 4, 128], dt.float32, tag="sq")
                srp = sq[:, 0:2]
                sip = sq[:, 2:4]
                for mt in range(2):
                    for kb in range(2):
                        nc.tensor.matmul(srp[:, mt], mrt_sb[:, kb, mt, :], feats[:, kb],
                                         start=(kb == 0), stop=(kb == 1))
                    for kb in range(2):
                        nc.tensor.matmul(sip[:, mt], mit_sb[:, kb, mt, :], feats[:, kb],
                                         start=(kb == 0), stop=(kb == 1))

                if debug:
                    nc.sync.dma_start(out=dbg_p2f[:, :, :], in_=p2full)
                    nc.sync.dma_start(out=dbg_feats[:, :, :], in_=feats)

                probs = tail.tile([128, 2, 128], dt.bfloat16, tag="probs")
                for mt in range(2):
                    t1 = tail.tile([128, 128], dt.float32, tag=f"sq_r{mt}")
                    nc.scalar.activation(t1, srp[:, mt], AF.Square)
                    t2s = tail.tile([128, 128], dt.float32, tag=f"sq_i{mt}")
                    nc.scalar.activation(t2s, sip[:, mt], AF.Square)
                    nc.vector.tensor_tensor(probs[:, mt], t1, t2s, ALU.add)

                qt = psumT.tile([8, 2, 128], dt.float32, tag="qt")
                qp = qt[:, 0]
                tp = qt[0:1, 1]
                for kb in range(2):
                    nc.tensor.matmul(qp, zext_sb[:, kb, 0:8], probs[:, kb],
                                     start=(kb == 0), stop=(kb == 1))
                for kb in range(2):
                    nc.tensor.matmul(tp, zext_sb[:, kb, 8:9], probs[:, kb],
                                     start=(kb == 0), stop=(kb == 1))

                recip = tail.tile([1, 128], dt.float32, tag="recip")
                nc.vector.reciprocal_approx_fast(recip, tp)
                recip_bf = tail.tile([1, 128], dt.bfloat16, tag="recip_bf")
                nc.vector.tensor_copy(out=recip_bf, in_=recip)
                bcp = psumT.tile([8, 128], dt.float32, tag="bcp")
                nc.tensor.matmul(bcp, ones18, recip_bf, start=True, stop=True)
                bc_sb = tail.tile([8, 128], dt.bfloat16, tag="bc_sb")
                nc.scalar.activation(bc_sb, bcp, AF.Copy)

                qn = tail.tile([8, 128], dt.bfloat16, tag="qn")
                nc.vector.tensor_tensor(qn, qp, bc_sb, ALU.mult)

                z1p = psumT.tile([128, 128], dt.float32, tag="z1p")
                nc.tensor.matmul(z1p, p1t_sb, qn, start=True, stop=True)
                z1 = tail.tile([128, 128], dt.bfloat16, tag="z1")
                nc.scalar.activation(z1, z1p, AF.Relu, bias=pb1_sb[:, 0:1])

                z2p = psumT.tile([64, 128], dt.float32, tag="z2p")
                nc.tensor.matmul(z2p, p2t_sb, z1, start=True, stop=True)
                z2 = tail.tile([64, 128], dt.bfloat16, tag="z2")
                nc.scalar.activation(z2, z2p, AF.Relu, bias=pb2_sb[:, 0:1])

                z3p = psumT.tile([10, 128], dt.float32, tag="z3p")
                nc.tensor.matmul(z3p, p3t_sb, z2, start=True, stop=True)
                osb = tail.tile([10, 128], dt.float32, tag="osb")
                nc.vector.tensor_scalar_add(osb, z3p, pb3_sb[:, 0:1])
                nc.sync.dma_start(out=out[:, :], in_=osb)

    nc.finalize()
    return nc


def _get_nc():
    if "nc" not in _CACHE:
        _CACHE["nc"] = _build_bass()
    return _CACHE["nc"]


def kernel(**inputs) -> np.ndarray:
    from concourse.bass_utils import run_bass_kernel_spmd

    in_maps = _host_prep(inputs)
    nc = _get_nc()
    res = run_bass_kernel_spmd(nc, in_maps, core_ids=list(range(NCORES)),
                               trace=bool(_CACHE.get("trace")))
    _CACHE["last_result"] = res
    outs = [r["out"].T for r in res.results]  # each [128, 10]
    return np.ascontiguousarray(np.concatenate(outs, axis=0), dtype=np.float32)


# revision 98
# speedup vs baseline: 1.1197x; 1.1197x over previous
"""AmplitudeQuantumNet Trainium2 kernel (8-core data parallel).

Pipeline per core (128 samples), 8 chunks of 16 samples, conv2 emitted
LAG chunks behind conv1 so the PE never idles (keeps the HAM clock gate
at 2.4 GHz -- idle gaps re-throttle it to 1.2 GHz):
  conv1(1->32,3x3)+BN+bias -> relu+pool-x        [K=19 im2col matmul;
                                                  ACT odd-col extract + STT]
  pool-y + x-shift replication                   [vector max pair + contiguous
                                                  byte-shifted SBUF DMAs]
  conv2(32->64,3x3)+BN (parity-split dy)         [6 K=96 matmuls/subtile,
                                                  392-col free dim]
  4-way max (y-parity x pool-x)                  [one vector reduce, axis=XY]
  relu+bias scatter to fc layout                 [scalar ACT, (ypm,xp,s)
                                                  iteration: contiguous runs]
  fc(3136->256)+tanh                             [K=128 (c + 64*(yp//4)),
                                                  2x28 matmuls, contiguous rhs]
  quantum statevector sim                        [host-built 256x256 unitary,
                                                  real+imag bf16 matmuls]
  probs -> Z expvals (norm via unitarity) -> MLP [tiny matmuls]

conv2 parity trick: conv1-pooled rows are kept split by y-parity (E=even
pooled rows, O=odd).  conv2 output rows 2yy / 2yy+1 are computed as two
separate 3-matmul accumulations over (E, O) row windows, and maxpool-y
then reduces the two parities at equal yy -- no interleave copies needed.
x-shifts (+-1 col) for the 3x3 kernel are physical partition-block copies
made with single contiguous byte-shifted SBUF DMAs (pad cols absorb the
wrap).

DMA notes (this axon/VNC setup): each DMA completion semaphore fires
~4us after the transfer visibly ends, and triggers cost ~0.6-1us on the
issuing engine's queue, so: xim chunk loads are all pre-issued on the
sync ring ahead of the per-chunk shift DMAs, the big tail weights ride
the same ring right after them, and everything latency-critical intra-
chunk (pool-y max, rc copy) stays on compute engines.
"""

import sys

sys.path.insert(0, "/opt/trn_rl_repo")

import numpy as np
import ml_dtypes

BF16 = ml_dtypes.bfloat16

N_QUBITS = 8
Q_DEPTH = 10
DIM = 256
BN_EPS = 1e-5
B = 1024
NCORES = 8
B_CORE = B // NCORES  # 128
SUB = 4               # samples per pipeline subtile
NCH = 8               # chunks per core
CH = B_CORE // NCH    # 16 samples per chunk
SPC = CH // SUB       # 4 subtiles per chunk
JY = [0, 2, 1, 3]     # conv1 partition-block -> jy shift (pool pairs at +-64)

_CACHE = {}


# ---------------------------------------------------------------- host precompute
def _quantum_unitary(q_params):
    """256x256 complex matrix of the full circuit (H layer + 10x[RX layer + diag])."""
    bits = ((np.arange(DIM)[:, None] >> (N_QUBITS - 1 - np.arange(N_QUBITS))) & 1)
    ph = np.where(np.arange(N_QUBITS) % 2 == 0, 1j, np.exp(1j * np.pi / 4))
    diag = np.prod(np.power(ph[None, :], bits), axis=1)
    cz = np.ones(DIM)
    for i, j in [(0, 1), (2, 3), (4, 5), (6, 7), (1, 2), (3, 4), (5, 6)]:
        cz = cz * ((-1.0) ** (bits[:, i] * bits[:, j]))
    diagc = (diag * cz).astype(np.complex128)

    def app(M, U, w):
        M = M.reshape((2,) * N_QUBITS + (DIM,))
        M = np.moveaxis(M, w, 0)
        M = np.tensordot(U, M, axes=(1, 0))
        M = np.moveaxis(M, 0, w)
        return M.reshape(DIM, DIM)

    M = np.eye(DIM, dtype=np.complex128)
    H = np.array([[1.0, 1.0], [1.0, -1.0]]) / np.sqrt(2.0)
    for w in range(N_QUBITS):
        M = app(M, H, w)
    qw = np.asarray(q_params, np.float64).reshape(Q_DEPTH, N_QUBITS)
    X = np.array([[0.0, 1.0], [1.0, 0.0]])
    I2 = np.eye(2)
    for layer in range(Q_DEPTH):
        for w in range(N_QUBITS):
            t = qw[layer, w]
            U = np.cos(t / 2) * I2 - 1j * np.sin(t / 2) * X
            M = app(M, U, w)
        M = diagc[:, None] * M
    zsigns = (1 - 2 * bits).astype(np.float64)  # [256, 8]
    return M, zsigns


def _host_prep(inputs):
    f32 = np.float32
    x = np.asarray(inputs["x"], f32)  # [1024,1,28,28]

    inv1 = inputs["bn1_gamma"] / np.sqrt(inputs["bn1_var"] + BN_EPS)
    w1f = np.asarray(inputs["conv1_w"], f32) * inv1[:, None, None, None]
    b1f = (inputs["conv1_b"] - inputs["bn1_mean"]) * inv1 + inputs["bn1_beta"]
    inv2 = inputs["bn2_gamma"] / np.sqrt(inputs["bn2_var"] + BN_EPS)
    w2f = np.asarray(inputs["conv2_w"], f32) * inv2[:, None, None, None]
    b2f = (inputs["conv2_b"] - inputs["bn2_mean"]) * inv2 + inputs["bn2_beta"]

    # conv1 lhsT [19, 128]: rows (r6,dxc) + bias row; cols (jyblk, c)
    W1 = np.zeros((19, 128), f32)
    for blk in range(4):
        jy = JY[blk]
        for r6 in range(6):
            dy = r6 - jy
            if 0 <= dy <= 2:
                for dxc in range(3):
                    W1[r6 * 3 + dxc, blk * 32:(blk + 1) * 32] = w1f[:, 0, dy, dxc]
        W1[18, blk * 32:(blk + 1) * 32] = b1f
    W1 = np.ascontiguousarray(W1, f32).astype(BF16)

    # x im2col [1024, 19, 7, 28]: row p=(r6,dxc): xpad[s, 4q+r6, x+dxc]; row 18 = 1
    xp = np.zeros((B, 30, 30), f32)
    xp[:, 1:29, 1:29] = x[:, 0]
    xim = np.empty((B, 19, 7, 28), f32)
    for r6 in range(6):
        for dxc in range(3):
            xim[:, r6 * 3 + dxc] = xp[:, r6:r6 + 25:4, dxc:dxc + 28]
    xim[:, 18] = 1.0
    # per-core partition-major [19, 128, 7, 28]
    xim_cores = [
        np.ascontiguousarray(xim[i * B_CORE:(i + 1) * B_CORE].transpose(1, 0, 2, 3)).astype(BF16)
        for i in range(NCORES)
    ]

    # conv2 lhsT pair, parity-split.  E-tile blocks: [center(k=1), +1(k=0), -1(k=2)];
    # O-tile blocks: [+1(k=0), center(k=1), -1(k=2)].
    wt = w2f.transpose(1, 2, 0, 3)  # [c, dy, m, k]
    W2E = np.zeros((96, 3, 64), f32)
    W2O = np.zeros((96, 3, 64), f32)
    for blk, (ke, ko) in enumerate(zip([1, 0, 2], [0, 1, 2])):
        W2E[blk * 32:(blk + 1) * 32] = wt[:, :, :, ke]
        W2O[blk * 32:(blk + 1) * 32] = wt[:, :, :, ko]
    W2E = W2E.astype(BF16)
    W2O = W2O.astype(BF16)

    # fc lhsT [128, 28, 2, 128]: K-row = c + 64*g (g = yp//4), j = (yp%4... g*4+ypm)*7+xp
    # p2full[c + 64g, ypm*7+xp, s] = pooled[c, 4g+ypm, xp, s]
    v = np.asarray(inputs["fc_w"], f32).reshape(2, 128, 64, 7, 7)  # [mt, m, c, yp, xp]
    fcw = np.zeros((128, 28, 2, 128), f32)
    fcw[0:64, 0:28] = v[:, :, :, 0:4, :].transpose(2, 3, 4, 0, 1).reshape(64, 28, 2, 128)
    fcw[64:128, 0:21] = v[:, :, :, 4:7, :].transpose(2, 3, 4, 0, 1).reshape(64, 21, 2, 128)
    fcw = np.ascontiguousarray(fcw).astype(BF16)
    fcb = np.ascontiguousarray(np.asarray(inputs["fc_b"], f32).reshape(2, 128).T)  # [128, 2]

    M, zsigns = _quantum_unitary(np.asarray(inputs["q_params"], np.float64))
    # lhsT tiles [k128, kb2, mt2, m128]: value M[mt*128+m, kb*128+k]
    mrt = M.real.T.reshape(2, 128, 2, 128).transpose(1, 0, 2, 3)
    mit = M.imag.T.reshape(2, 128, 2, 128).transpose(1, 0, 2, 3)
    mrt = np.ascontiguousarray(mrt).astype(f32).astype(BF16)
    mit = np.ascontiguousarray(mit).astype(f32).astype(BF16)
    zext = np.ones((DIM, 9), np.float64)
    zext[:, :8] = zsigns
    zext = np.ascontiguousarray(zext.reshape(2, 128, 9).transpose(1, 0, 2)).astype(f32).astype(BF16)

    p1t = np.ascontiguousarray(np.asarray(inputs["p1_w"], f32).T).astype(BF16)  # [8,128]
    p2t = np.ascontiguousarray(np.asarray(inputs["p2_w"], f32).T).astype(BF16)  # [128,64]
    p3t = np.ascontiguousarray(np.asarray(inputs["p3_w"], f32).T).astype(BF16)  # [64,10]

    common = {
        "w1": W1, "w2e": W2E, "w2o": W2O, "fcw": fcw, "fcb": fcb,
        "mrt": mrt, "mit": mit, "zext": zext,
        "p1t": p1t, "p2t": p2t, "p3t": p3t,
        "cb2": np.asarray(b2f, f32).reshape(64, 1),
        "pb1": np.asarray(inputs["p1_b"], f32).reshape(128, 1),
        "pb2": np.asarray(inputs["p2_b"], f32).reshape(64, 1),
        "pb3": np.asarray(inputs["p3_b"], f32).reshape(10, 1),
    }
    in_maps = []
    for i in range(NCORES):
        m = dict(common)
        m["xim"] = xim_cores[i]
        in_maps.append(m)
    return in_maps


# ---------------------------------------------------------------- bass program
def _build_bass():
    import concourse.bacc as bacc
    import concourse.mybir as mybir
    import concourse.tile as tile

    dt = mybir.dt
    AF = mybir.ActivationFunctionType
    ALU = mybir.AluOpType
    AX = mybir.AxisListType

    nc = bacc.Bacc("TRN2", target_bir_lowering=False, debug=False,
                   num_devices=NCORES)
    xim = nc.dram_tensor("xim", [19, B_CORE, 7, 28], dt.bfloat16, kind="ExternalInput")
    w1 = nc.dram_tensor("w1", [19, 128], dt.bfloat16, kind="ExternalInput")
    w2e = nc.dram_tensor("w2e", [96, 3, 64], dt.bfloat16, kind="ExternalInput")
    w2o = nc.dram_tensor("w2o", [96, 3, 64], dt.bfloat16, kind="ExternalInput")
    fcw = nc.dram_tensor("fcw", [128, 28, 2, 128], dt.bfloat16, kind="ExternalInput")
    fcb = nc.dram_tensor("fcb", [128, 2], dt.float32, kind="ExternalInput")
    mrt = nc.dram_tensor("mrt", [128, 2, 2, 128], dt.bfloat16, kind="ExternalInput")
    mit = nc.dram_tensor("mit", [128, 2, 2, 128], dt.bfloat16, kind="ExternalInput")
    zext = nc.dram_tensor("zext", [128, 2, 9], dt.bfloat16, kind="ExternalInput")
    p1t = nc.dram_tensor("p1t", [8, 128], dt.bfloat16, kind="ExternalInput")
    p2t = nc.dram_tensor("p2t", [128, 64], dt.bfloat16, kind="ExternalInput")
    p3t = nc.dram_tensor("p3t", [64, 10], dt.bfloat16, kind="ExternalInput")
    cb2 = nc.dram_tensor("cb2", [64, 1], dt.float32, kind="ExternalInput")
    pb1 = nc.dram_tensor("pb1", [128, 1], dt.float32, kind="ExternalInput")
    pb2 = nc.dram_tensor("pb2", [64, 1], dt.float32, kind="ExternalInput")
    pb3 = nc.dram_tensor("pb3", [10, 1], dt.float32, kind="ExternalInput")
    out = nc.dram_tensor("out", [10, B_CORE], dt.float32, kind="ExternalOutput")
    debug = bool(_CACHE.get("debug"))
    if debug:
        dbg_et = nc.dram_tensor("dbg_et", [96, CH, 8, 16], dt.bfloat16, kind="ExternalOutput")
        dbg_ot = nc.dram_tensor("dbg_ot", [96, CH, 8, 16], dt.bfloat16, kind="ExternalOutput")
        dbg_p2f = nc.dram_tensor("dbg_p2f", [128, 28, B_CORE], dt.bfloat16, kind="ExternalOutput")
        dbg_feats = nc.dram_tensor("dbg_feats", [128, 2, 128], dt.bfloat16, kind="ExternalOutput")
        dbg_p1c = nc.dram_tensor("dbg_p1c", [128, CH, 7, 14], dt.bfloat16, kind="ExternalOutput")
        dbg_t3 = nc.dram_tensor("dbg_t3", [64, SUB, 7, 7], dt.bfloat16, kind="ExternalOutput")
        dbg_pe = nc.dram_tensor("dbg_pe", [64, 2, 448], dt.float32, kind="ExternalOutput")

    NEO = CH * 8 * 16  # flat elements per partition of an E/O tile

    with tile.TileContext(nc) as tc:
        with tc.tile_pool(name="singles", bufs=1) as singles:
            # conv1-critical loads on the sync HWDGE ring, first.
            w1_sb = singles.tile([19, 128], dt.bfloat16, tag="w1")
            nc.sync.dma_start(out=w1_sb, in_=w1[:, :], single_packet=True)
            # everything small on the scalar HWDGE ring.
            w2e_sb = singles.tile([96, 3, 64], dt.bfloat16, tag="w2e")
            nc.scalar.dma_start(out=w2e_sb, in_=w2e[:, :, :])
            w2o_sb = singles.tile([96, 3, 64], dt.bfloat16, tag="w2o")
            nc.scalar.dma_start(out=w2o_sb, in_=w2o[:, :, :])
            cb2_sb = singles.tile([64, 1], dt.float32, tag="cb2")
            nc.scalar.dma_start(out=cb2_sb, in_=cb2[:, :])
            fcb_sb = singles.tile([128, 2], dt.float32, tag="fcb")
            nc.scalar.dma_start(out=fcb_sb, in_=fcb[:, :])
            p1t_sb = singles.tile([8, 128], dt.bfloat16, tag="p1t")
            nc.scalar.dma_start(out=p1t_sb, in_=p1t[:, :])
            p2t_sb = singles.tile([128, 64], dt.bfloat16, tag="p2t")
            nc.scalar.dma_start(out=p2t_sb, in_=p2t[:, :])
            p3t_sb = singles.tile([64, 10], dt.bfloat16, tag="p3t")
            nc.scalar.dma_start(out=p3t_sb, in_=p3t[:, :])
            pb1_sb = singles.tile([128, 1], dt.float32, tag="pb1")
            nc.scalar.dma_start(out=pb1_sb, in_=pb1[:, :])
            pb2_sb = singles.tile([64, 1], dt.float32, tag="pb2")
            nc.scalar.dma_start(out=pb2_sb, in_=pb2[:, :])
            pb3_sb = singles.tile([10, 1], dt.float32, tag="pb3")
            nc.scalar.dma_start(out=pb3_sb, in_=pb3[:, :])

            # big tail weights: tiles now, DMAs emitted mid-conv-loop (sync ring)
            fcw_sb = singles.tile([128, 28, 2, 128], dt.bfloat16, tag="fcw")
            mrt_sb = singles.tile([128, 2, 2, 128], dt.bfloat16, tag="mrt")
            mit_sb = singles.tile([128, 2, 2, 128], dt.bfloat16, tag="mit")
            zext_sb = singles.tile([128, 2, 9], dt.bfloat16, tag="zext")

            # fc input [c + 64*(yp//4), (yp%4)*7+xp, s] -- sample-innermost so
            # the fc matmul rhs is contiguous (strided rhs slows the PE ~4x)
            p2full = singles.tile([128, 28, B_CORE], dt.bfloat16, tag="p2full")
            # j=21..27 of the upper half is never written; fc reads it with
            # zero weights, so it must at least be finite.
            nc.gpsimd.memset(p2full[64:128, 21:28, :], 0.0)

            ones18 = singles.tile([1, 8], dt.bfloat16, tag="ones18")
            nc.gpsimd.memset(ones18, 1.0)

            # per-chunk conv2 input tiles (persistent; only pads need zeroing,
            # emitted per-chunk inside the loop to keep engine queues clear)
            Et = [singles.tile([96, CH, 8, 16], dt.bfloat16, tag=f"Et{ci}",
                               name=f"Et{ci}") for ci in range(NCH)]
            Ot = [singles.tile([96, CH, 8, 16], dt.bfloat16, tag=f"Ot{ci}",
                               name=f"Ot{ci}") for ci in range(NCH)]

            with tc.tile_pool(name="ximp", bufs=8) as ximpool, \
                 tc.tile_pool(name="oddp", bufs=6) as oddpool, \
                 tc.tile_pool(name="p1cp", bufs=4) as p1cpool, \
                 tc.tile_pool(name="rcp", bufs=4) as rcpool, \
                 tc.tile_pool(name="t3p", bufs=6) as t3pool, \
                 tc.tile_pool(name="ps1", bufs=2, space="PSUM") as psum1, \
                 tc.tile_pool(name="ps2", bufs=2, space="PSUM") as psum2:
                p1cs = {}

                def emit_pads(ci):
                    # zero only the pad regions of Et/Ot (rest is overwritten)
                    ef = Et[ci].rearrange("p a b c -> p (a b c)")
                    of = Ot[ci].rearrange("p a b c -> p (a b c)")
                    nc.gpsimd.memset(Et[ci][0:32, :, :, 14:16], 0.0)
                    nc.gpsimd.memset(Et[ci][0:32, :, 7:8, 0:14], 0.0)
                    nc.gpsimd.memset(ef[32:64, 0:1], 0.0)
                    nc.gpsimd.memset(ef[64:96, NEO - 1:NEO], 0.0)
                    nc.gpsimd.memset(ef[64:96, NEO // 2 - 1:NEO // 2], 0.0)
                    nc.gpsimd.memset(Ot[ci][32:64, :, :, 14:16], 0.0)
                    nc.gpsimd.memset(Ot[ci][32:64, :, 0:1, 0:14], 0.0)
                    nc.gpsimd.memset(of[0:32, 0:1], 0.0)
                    nc.gpsimd.memset(of[64:96, NEO - 1:NEO], 0.0)
                    nc.gpsimd.memset(of[64:96, NEO // 2 - 1:NEO // 2], 0.0)

                def emit_conv1(ci, tt0, tt1):
                    xim_sb = xim_sbs[ci]
                    if tt0 == 0:
                        p1c = p1cpool.tile([128, CH, 7, 14], dt.bfloat16, tag="p1c")
                        p1cs[ci] = p1c
                    else:
                        p1c = p1cs[ci]
                    for tt in range(tt0, tt1):
                        c1p = psum1.tile([128, SUB // 2, 512], dt.float32, tag="c1p")
                        for sh in range(SUB // 2):
                            s0 = tt * SUB + sh * 2
                            if ci == 0 and tt < 2:
                                xsrc = xim0a[:, s0:s0 + 2]
                            else:
                                xsrc = xim_sb[:, s0:s0 + 2]
                            nc.tensor.matmul(
                                c1p[:, sh, 0:392].rearrange(
                                    "p (s q x) -> p s q x", s=2, q=7, x=28),
                                w1_sb, xsrc, start=True, stop=True)
                        c1v = c1p[:, :, 0:392].rearrange(
                            "p h (s q xp two) -> p h s q xp two", s=2, q=7, xp=14, two=2)
                        odd1 = oddpool.tile([128, SUB, 7, 14], dt.bfloat16, tag="odd1")
                        o1v = odd1.rearrange("p (h s) q xp -> p h s q xp", h=SUB // 2)
                        nc.scalar.activation(o1v, c1v[:, :, :, :, :, 1], AF.Copy)
                        nc.vector.scalar_tensor_tensor(
                            p1c[:, tt * SUB:(tt + 1) * SUB].rearrange(
                                "p (h s) q xp -> p h s q xp", h=SUB // 2),
                            c1v[:, :, :, :, :, 0], 0.0, o1v, ALU.max, ALU.max)

                def emit_pooly(ci):
                    p1c = p1cs[ci]
                    rc = rcpool.tile([64, CH, 7, 14], dt.bfloat16, tag="rc")
                    nc.vector.tensor_copy(out=rc, in_=p1c[64:128])
                    # E rows 0..6 = even pooled rows; O rows 1..7 = odd pooled rows
                    nc.vector.tensor_tensor(
                        Et[ci][0:32, :, 0:7, 0:14], p1c[0:32], rc[0:32], ALU.max)
                    nc.vector.tensor_tensor(
                        Ot[ci][32:64, :, 1:8, 0:14], p1c[32:64], rc[32:64], ALU.max)
                    ef = Et[ci].rearrange("p a b c -> p (a b c)")
                    of = Ot[ci].rearrange("p a b c -> p (a b c)")
                    # +-1 column shifts as single contiguous byte-shifted copies;
                    # sync ring (xims are pre-issued, so nothing queues behind these)
                    nc.sync.dma_start(out=ef[32:64, 1:NEO], in_=ef[0:32, 0:NEO - 1])
                    nc.sync.dma_start(out=ef[64:96, 0:NEO - 1], in_=ef[0:32, 1:NEO])
                    nc.sync.dma_start(out=of[0:32, 1:NEO], in_=of[32:64, 0:NEO - 1])
                    nc.sync.dma_start(out=of[64:96, 0:NEO - 1], in_=of[32:64, 1:NEO])

                def emit_conv2(ci):
                    for tt in range(SPC):
                        s0 = tt * SUB
                        ps = psum2.tile([64, 2, 512], dt.float32, tag="c2p")
                        pe = ps[:, 0, 0:448].rearrange(
                            "p (s y x) -> p s y x", s=SUB, y=7, x=16)[:, :, :, 0:14]
                        po = ps[:, 1, 0:448].rearrange(
                            "p (s y x) -> p s y x", s=SUB, y=7, x=16)[:, :, :, 0:14]
                        Ev = Et[ci][:, s0:s0 + SUB]
                        Ov = Ot[ci][:, s0:s0 + SUB]
                        # even out rows: W[0]*O[yy-1] + W[1]*E[yy] + W[2]*O[yy]
                        nc.tensor.matmul(pe, w2o_sb[:, 0, :], Ov[:, :, 0:7, 0:14],
                                         start=True, stop=False)
                        nc.tensor.matmul(pe, w2e_sb[:, 1, :], Ev[:, :, 0:7, 0:14],
                                         start=False, stop=False)
                        nc.tensor.matmul(pe, w2o_sb[:, 2, :], Ov[:, :, 1:8, 0:14],
                                         start=False, stop=True)
                        # odd out rows: W[0]*E[yy] + W[1]*O[yy] + W[2]*E[yy+1]
                        nc.tensor.matmul(po, w2e_sb[:, 0, :], Ev[:, :, 0:7, 0:14],
                                         start=True, stop=False)
                        nc.tensor.matmul(po, w2o_sb[:, 1, :], Ov[:, :, 1:8, 0:14],
                                         start=False, stop=False)
                        nc.tensor.matmul(po, w2e_sb[:, 2, :], Ev[:, :, 1:8, 0:14],
                                         start=False, stop=True)
                        # fused maxpool 2x2: max over (y-parity, x-pair); junk
                        # col pair (14,15) excluded from the input AP
                        t3 = t3pool.tile([64, SUB, 7, 7], dt.bfloat16, tag="t3")
                        rin = ps[:, :, 0:448].rearrange(
                            "p par (s y xh two) -> p s y xh par two",
                            s=SUB, y=7, xh=8, two=2)[:, :, :, 0:7]
                        nc.vector.tensor_reduce(t3, rin, axis=AX.XY, op=ALU.max)
                        if debug and ci == 0 and tt == 0:
                            nc.sync.dma_start(out=dbg_t3[:, :, :, :], in_=t3)
                            pecp = t3pool.tile([64, 2, 448], dt.float32, tag="pecp")
                            nc.scalar.activation(pecp, ps[:, :, 0:448], AF.Copy)
                            nc.sync.dma_start(out=dbg_pe[:, :, :], in_=pecp)
                        # relu + conv2 bias, scatter to fc layout
                        gs = ci * CH + s0
                        # iterate (ypm, xp, s) so dst writes are contiguous
                        # 4-sample runs instead of stride-128 singles
                        dst0 = p2full[0:64].rearrange(
                            "p (ypm xp) s -> p ypm xp s", ypm=4)[:, :, :, gs:gs + SUB]
                        dst1 = p2full[64:128, 0:21, :].rearrange(
                            "p (ypm xp) s -> p ypm xp s", ypm=3)[:, :, :, gs:gs + SUB]
                        if ci == NCH - 1 and tt == SPC - 1:
                            # final subtile: emit on vector right after its own
                            # reduce so fc isn't gated on a scalar-queue hop
                            nc.vector.tensor_scalar(
                                dst0, t3[:, :, 0:4, :].rearrange("p s y x -> p y x s"),
                                cb2_sb[:, 0:1], 0.0, ALU.add, ALU.max)
                            nc.vector.tensor_scalar(
                                dst1, t3[:, :, 4:7, :].rearrange("p s y x -> p y x s"),
                                cb2_sb[:, 0:1], 0.0, ALU.add, ALU.max)
                        else:
                            nc.scalar.activation(
                                dst0, t3[:, :, 0:4, :].rearrange("p s y x -> p y x s"),
                                AF.Relu, bias=cb2_sb[:, 0:1])
                            nc.scalar.activation(
                                dst1, t3[:, :, 4:7, :].rearrange("p s y x -> p y x s"),
                                AF.Relu, bias=cb2_sb[:, 0:1])

                LAG = 5
                xim0a = ximpool.tile([19, 2 * SUB, 7, 28], dt.bfloat16, tag="xim0a")
                nc.sync.dma_start(out=xim0a, in_=xim[:, 0:2 * SUB, :, :],
                                  single_packet=True)
                xim_sbs = {}
                for ci in range(NCH):
                    xim_sbs[ci] = ximpool.tile([19, CH, 7, 28], dt.bfloat16,
                                               tag="xim_sb", name=f"xim{ci}")
                    nc.sync.dma_start(out=xim_sbs[ci],
                                      in_=xim[:, ci * CH:(ci + 1) * CH, :, :])
                nc.sync.dma_start(out=fcw_sb, in_=fcw[:, :, :, :])
                nc.sync.dma_start(out=mrt_sb, in_=mrt[:, :, :, :])
                nc.sync.dma_start(out=mit_sb, in_=mit[:, :, :, :])
                nc.sync.dma_start(out=zext_sb, in_=zext[:, :, :])
                emit_pads(0)
                emit_pads(1)
                for ci in range(NCH):
                    emit_conv1(ci, 0, SPC)
                    if ci + 2 < NCH:
                        emit_pads(ci + 2)
                    emit_pooly(ci)
                    if ci >= LAG:
                        emit_conv2(ci - LAG)
                for ci in range(NCH - LAG, NCH):
                    emit_conv2(ci)
                if debug:
                    nc.sync.dma_start(out=dbg_et[:, :, :, :], in_=Et[0])
                    nc.sync.dma_start(out=dbg_ot[:, :, :, :], in_=Ot[0])

            # ---------------- dense tail ----------------
            with tc.tile_pool(name="tail", bufs=1) as tail, \
                 tc.tile_pool(name="psumT", bufs=1, space="PSUM") as psumT:
                fp = psumT.tile([128, 2, 128], dt.float32, tag="fp")
                feats = tail.tile([128, 2, 128], dt.bfloat16, tag="feats")
                for mt in range(2):
                    for j in range(28):
                        nc.tensor.matmul(
                            fp[:, mt], fcw_sb[:, j, mt, :], p2full[:, j, :],
                            start=(j == 0), stop=(j == 27))
                    # tanh(mt) overlaps the next mt's matmuls on scalar
                    nc.scalar.activation(feats[:, mt], fp[:, mt], AF.Tanh,
                                         bias=fcb_sb[:, mt:mt + 1])

                sq = psumT.tile([128, 4, 128], dt.float32, tag="sq")
                srp = sq[:, 0:2]
                sip = sq[:, 2:4]
                for mt in range(2):
                    for kb in range(2):
                        nc.tensor.matmul(srp[:, mt], mrt_sb[:, kb, mt, :], feats[:, kb],
                                         start=(kb == 0), stop=(kb == 1))
                    for kb in range(2):
                        nc.tensor.matmul(sip[:, mt], mit_sb[:, kb, mt, :], feats[:, kb],
                                         start=(kb == 0), stop=(kb == 1))

                if debug:
                    nc.sync.dma_start(out=dbg_p2f[:, :, :], in_=p2full)
                    nc.sync.dma_start(out=dbg_feats[:, :, :], in_=feats)

                probs = tail.tile([128, 2, 128], dt.bfloat16, tag="probs")
                for mt in range(2):
                    t1 = tail.tile([128, 128], dt.float32, tag=f"sq_r{mt}")
                    nc.scalar.activation(t1, srp[:, mt], AF.Square)
                    t2s = tail.tile([128, 128], dt.float32, tag=f"sq_i{mt}")
                    nc.scalar.activation(t2s, sip[:, mt], AF.Square)
                    nc.vector.tensor_tensor(probs[:, mt], t1, t2s, ALU.add)

                qt = psumT.tile([8, 2, 128], dt.float32, tag="qt")
                qp = qt[:, 0]
                tp = qt[0:1, 1]
                for kb in range(2):
                    nc.tensor.matmul(qp, zext_sb[:, kb, 0:8], probs[:, kb],
                                     start=(kb == 0), stop=(kb == 1))
                for kb in range(2):
                    nc.tensor.matmul(tp, zext_sb[:, kb, 8:9], probs[:, kb],
                                     start=(kb == 0), stop=(kb == 1))

                recip = tail.tile([1, 128], dt.float32, tag="recip")
                nc.vector.reciprocal_approx_fast(recip, tp)
                recip_bf = tail.tile([1, 128], dt.bfloat16, tag="recip_bf")
                nc.vector.tensor_copy(out=recip_bf, in_=recip)
                bcp = psumT.tile([8, 128], dt.float32, tag="bcp")
                nc.tensor.matmul(bcp, ones18, recip_bf, start=True, stop=True)
                bc_sb = tail.tile([8, 128], dt.bfloat16, tag="bc_sb")
                nc.scalar.activation(bc_sb, bcp, AF.Copy)

                qn = tail.tile([8, 128], dt.bfloat16, tag="qn")
                nc.vector.tensor_tensor(qn, qp, bc_sb, ALU.mult)

                z1p = psumT.tile([128, 128], dt.float32, tag="z1p")
                nc.tensor.matmul(z1p, p1t_sb, qn, start=True, stop=True)
                z1 = tail.tile([128, 128], dt.bfloat16, tag="z1")
                nc.scalar.activation(z1, z1p, AF.Relu, bias=pb1_sb[:, 0:1])

                z2p = psumT.tile([64, 128], dt.float32, tag="z2p")
                nc.tensor.matmul(z2p, p2t_sb, z1, start=True, stop=True)
                z2 = tail.tile([64, 128], dt.bfloat16, tag="z2")
                nc.scalar.activation(z2, z2p, AF.Relu, bias=pb2_sb[:, 0:1])

                z3p = psumT.tile([10, 128], dt.float32, tag="z3p")
                nc.tensor.matmul(z3p, p3t_sb, z2, start=True, stop=True)
                osb = tail.tile([10, 128], dt.float32, tag="osb")
                nc.vector.tensor_scalar_add(osb, z3p, pb3_sb[:, 0:1])
                nc.sync.dma_start(out=out[:, :], in_=osb)

    nc.finalize()
    return nc


def _get_nc():
    if "nc" not in _CACHE:
        _CACHE["nc"] = _build_bass()
    return _CACHE["nc"]


def kernel(**inputs) -> np.ndarray:
    from concourse.bass_utils import run_bass_kernel_spmd

    in_maps = _host_prep(inputs)
    nc = _get_nc()
    res = run_bass_kernel_spmd(nc, in_maps, core_ids=list(range(NCORES)),
                               trace=bool(_CACHE.get("trace")))
    _CACHE["last_result"] = res
    outs = [r["out"].T for r in res.results]  # each [128, 10]
    return np.ascontiguousarray(np.concatenate(outs, axis=0), dtype=np.float32)


# revision 101
# speedup vs baseline: 1.1359x; 1.0145x over previous
"""AmplitudeQuantumNet Trainium2 kernel (8-core data parallel).

Pipeline per core (128 samples), 8 chunks of 16 samples, conv2 emitted
LAG chunks behind conv1 so the PE never idles (keeps the HAM clock gate
at 2.4 GHz -- idle gaps re-throttle it to 1.2 GHz):
  conv1(1->32,3x3)+BN+bias -> relu+pool-x        [K=19 im2col matmul;
                                                  ACT odd-col extract + STT]
  pool-y + x-shift replication                   [vector max pair + contiguous
                                                  byte-shifted SBUF DMAs]
  conv2(32->64,3x3)+BN (parity-split dy)         [6 K=96 matmuls/subtile,
                                                  392-col free dim]
  4-way max (y-parity x pool-x)                  [one vector reduce, axis=XY]
  relu+bias scatter to fc layout                 [scalar ACT, (ypm,xp,s)
                                                  iteration: contiguous runs]
  fc(3136->256)+tanh                             [K=128 (c + 64*(yp//4)),
                                                  2x28 matmuls, contiguous rhs]
  quantum statevector sim                        [host-built 256x256 unitary,
                                                  real+imag bf16 matmuls]
  probs -> Z expvals (norm via unitarity) -> MLP [tiny matmuls]

conv2 parity trick: conv1-pooled rows are kept split by y-parity (E=even
pooled rows, O=odd).  conv2 output rows 2yy / 2yy+1 are computed as two
separate 3-matmul accumulations over (E, O) row windows, and maxpool-y
then reduces the two parities at equal yy -- no interleave copies needed.
x-shifts (+-1 col) for the 3x3 kernel are physical partition-block copies
made with single contiguous byte-shifted SBUF DMAs (pad cols absorb the
wrap).

DMA notes (this axon/VNC setup): each DMA completion semaphore fires
~4us after the transfer visibly ends, and triggers cost ~0.6-1us on the
issuing engine's queue, so: xim chunk loads are all pre-issued on the
sync ring ahead of the per-chunk shift DMAs, the big tail weights ride
the same ring right after them, and everything latency-critical intra-
chunk (pool-y max, rc copy) stays on compute engines.
"""

import sys

sys.path.insert(0, "/opt/trn_rl_repo")

import numpy as np
import ml_dtypes

BF16 = ml_dtypes.bfloat16

N_QUBITS = 8
Q_DEPTH = 10
DIM = 256
BN_EPS = 1e-5
B = 1024
NCORES = 8
B_CORE = B // NCORES  # 128
SUB = 4               # samples per pipeline subtile
NCH = 8               # chunks per core
CH = B_CORE // NCH    # 16 samples per chunk
SPC = CH // SUB       # 4 subtiles per chunk
JY = [0, 2, 1, 3]     # conv1 partition-block -> jy shift (pool pairs at +-64)

_CACHE = {}


# ---------------------------------------------------------------- host precompute
def _quantum_unitary(q_params):
    """256x256 complex matrix of the full circuit (H layer + 10x[RX layer + diag])."""
    bits = ((np.arange(DIM)[:, None] >> (N_QUBITS - 1 - np.arange(N_QUBITS))) & 1)
    ph = np.where(np.arange(N_QUBITS) % 2 == 0, 1j, np.exp(1j * np.pi / 4))
    diag = np.prod(np.power(ph[None, :], bits), axis=1)
    cz = np.ones(DIM)
    for i, j in [(0, 1), (2, 3), (4, 5), (6, 7), (1, 2), (3, 4), (5, 6)]:
        cz = cz * ((-1.0) ** (bits[:, i] * bits[:, j]))
    diagc = (diag * cz).astype(np.complex128)

    def app(M, U, w):
        M = M.reshape((2,) * N_QUBITS + (DIM,))
        M = np.moveaxis(M, w, 0)
        M = np.tensordot(U, M, axes=(1, 0))
        M = np.moveaxis(M, 0, w)
        return M.reshape(DIM, DIM)

    M = np.eye(DIM, dtype=np.complex128)
    H = np.array([[1.0, 1.0], [1.0, -1.0]]) / np.sqrt(2.0)
    for w in range(N_QUBITS):
        M = app(M, H, w)
    qw = np.asarray(q_params, np.float64).reshape(Q_DEPTH, N_QUBITS)
    X = np.array([[0.0, 1.0], [1.0, 0.0]])
    I2 = np.eye(2)
    for layer in range(Q_DEPTH):
        for w in range(N_QUBITS):
            t = qw[layer, w]
            U = np.cos(t / 2) * I2 - 1j * np.sin(t / 2) * X
            M = app(M, U, w)
        M = diagc[:, None] * M
    zsigns = (1 - 2 * bits).astype(np.float64)  # [256, 8]
    return M, zsigns


def _host_prep(inputs):
    f32 = np.float32
    x = np.asarray(inputs["x"], f32)  # [1024,1,28,28]

    inv1 = inputs["bn1_gamma"] / np.sqrt(inputs["bn1_var"] + BN_EPS)
    w1f = np.asarray(inputs["conv1_w"], f32) * inv1[:, None, None, None]
    b1f = (inputs["conv1_b"] - inputs["bn1_mean"]) * inv1 + inputs["bn1_beta"]
    inv2 = inputs["bn2_gamma"] / np.sqrt(inputs["bn2_var"] + BN_EPS)
    w2f = np.asarray(inputs["conv2_w"], f32) * inv2[:, None, None, None]
    b2f = (inputs["conv2_b"] - inputs["bn2_mean"]) * inv2 + inputs["bn2_beta"]

    # conv1 lhsT [19, 128]: rows (r6,dxc) + bias row; cols (jyblk, c)
    W1 = np.zeros((19, 128), f32)
    for blk in range(4):
        jy = JY[blk]
        for r6 in range(6):
            dy = r6 - jy
            if 0 <= dy <= 2:
                for dxc in range(3):
                    W1[r6 * 3 + dxc, blk * 32:(blk + 1) * 32] = w1f[:, 0, dy, dxc]
        W1[18, blk * 32:(blk + 1) * 32] = b1f
    W1 = np.ascontiguousarray(W1, f32).astype(BF16)

    # x im2col [1024, 19, 7, 28]: row p=(r6,dxc): xpad[s, 4q+r6, x+dxc]; row 18 = 1
    xp = np.zeros((B, 30, 30), f32)
    xp[:, 1:29, 1:29] = x[:, 0]
    xim = np.empty((B, 19, 7, 28), f32)
    for r6 in range(6):
        for dxc in range(3):
            xim[:, r6 * 3 + dxc] = xp[:, r6:r6 + 25:4, dxc:dxc + 28]
    xim[:, 18] = 1.0
    # per-core partition-major [19, 128, 7, 28]
    xim_cores = [
        np.ascontiguousarray(xim[i * B_CORE:(i + 1) * B_CORE].transpose(1, 0, 2, 3)).astype(BF16)
        for i in range(NCORES)
    ]

    # conv2 lhsT pair, parity-split.  E-tile blocks: [center(k=1), +1(k=0), -1(k=2)];
    # O-tile blocks: [+1(k=0), center(k=1), -1(k=2)].
    wt = w2f.transpose(1, 2, 0, 3)  # [c, dy, m, k]
    W2E = np.zeros((96, 3, 64), f32)
    W2O = np.zeros((96, 3, 64), f32)
    for blk, (ke, ko) in enumerate(zip([1, 0, 2], [0, 1, 2])):
        W2E[blk * 32:(blk + 1) * 32] = wt[:, :, :, ke]
        W2O[blk * 32:(blk + 1) * 32] = wt[:, :, :, ko]
    W2E = W2E.astype(BF16)
    W2O = W2O.astype(BF16)

    # fc lhsT [128, 28, 2, 128]: K-row = c + 64*g (g = yp//4), j = (yp%4... g*4+ypm)*7+xp
    # p2full[c + 64g, ypm*7+xp, s] = pooled[c, 4g+ypm, xp, s]
    v = np.asarray(inputs["fc_w"], f32).reshape(2, 128, 64, 7, 7)  # [mt, m, c, yp, xp]
    fcw = np.zeros((128, 28, 2, 128), f32)
    fcw[0:64, 0:28] = v[:, :, :, 0:4, :].transpose(2, 3, 4, 0, 1).reshape(64, 28, 2, 128)
    fcw[64:128, 0:21] = v[:, :, :, 4:7, :].transpose(2, 3, 4, 0, 1).reshape(64, 21, 2, 128)
    fcw = np.ascontiguousarray(fcw).astype(BF16)
    fcb = np.ascontiguousarray(np.asarray(inputs["fc_b"], f32).reshape(2, 128).T)  # [128, 2]

    M, zsigns = _quantum_unitary(np.asarray(inputs["q_params"], np.float64))
    # lhsT tiles [k128, kb2, mt2, m128]: value M[mt*128+m, kb*128+k]
    mrt = M.real.T.reshape(2, 128, 2, 128).transpose(1, 0, 2, 3)
    mit = M.imag.T.reshape(2, 128, 2, 128).transpose(1, 0, 2, 3)
    mrt = np.ascontiguousarray(mrt).astype(f32).astype(BF16)
    mit = np.ascontiguousarray(mit).astype(f32).astype(BF16)
    zext = np.ones((DIM, 9), np.float64)
    zext[:, :8] = zsigns
    zext = np.ascontiguousarray(zext.reshape(2, 128, 9).transpose(1, 0, 2)).astype(f32).astype(BF16)

    p1t = np.ascontiguousarray(np.asarray(inputs["p1_w"], f32).T).astype(BF16)  # [8,128]
    p2t = np.ascontiguousarray(np.asarray(inputs["p2_w"], f32).T).astype(BF16)  # [128,64]
    p3t = np.ascontiguousarray(np.asarray(inputs["p3_w"], f32).T).astype(BF16)  # [64,10]

    common = {
        "w1": W1, "w2e": W2E, "w2o": W2O, "fcw": fcw, "fcb": fcb,
        "mrt": mrt, "mit": mit, "zext": zext,
        "p1t": p1t, "p2t": p2t, "p3t": p3t,
        "cb2": np.asarray(b2f, f32).reshape(64, 1),
        "pb1": np.asarray(inputs["p1_b"], f32).reshape(128, 1),
        "pb2": np.asarray(inputs["p2_b"], f32).reshape(64, 1),
        "pb3": np.asarray(inputs["p3_b"], f32).reshape(10, 1),
    }
    in_maps = []
    for i in range(NCORES):
        m = dict(common)
        m["xim"] = xim_cores[i]
        in_maps.append(m)
    return in_maps


# ---------------------------------------------------------------- bass program
def _build_bass():
    import concourse.bacc as bacc
    import concourse.mybir as mybir
    import concourse.tile as tile

    dt = mybir.dt
    AF = mybir.ActivationFunctionType
    ALU = mybir.AluOpType
    AX = mybir.AxisListType

    nc = bacc.Bacc("TRN2", target_bir_lowering=False, debug=False,
                   num_devices=NCORES)
    xim = nc.dram_tensor("xim", [19, B_CORE, 7, 28], dt.bfloat16, kind="ExternalInput")
    w1 = nc.dram_tensor("w1", [19, 128], dt.bfloat16, kind="ExternalInput")
    w2e = nc.dram_tensor("w2e", [96, 3, 64], dt.bfloat16, kind="ExternalInput")
    w2o = nc.dram_tensor("w2o", [96, 3, 64], dt.bfloat16, kind="ExternalInput")
    fcw = nc.dram_tensor("fcw", [128, 28, 2, 128], dt.bfloat16, kind="ExternalInput")
    fcb = nc.dram_tensor("fcb", [128, 2], dt.float32, kind="ExternalInput")
    mrt = nc.dram_tensor("mrt", [128, 2, 2, 128], dt.bfloat16, kind="ExternalInput")
    mit = nc.dram_tensor("mit", [128, 2, 2, 128], dt.bfloat16, kind="ExternalInput")
    zext = nc.dram_tensor("zext", [128, 2, 9], dt.bfloat16, kind="ExternalInput")
    p1t = nc.dram_tensor("p1t", [8, 128], dt.bfloat16, kind="ExternalInput")
    p2t = nc.dram_tensor("p2t", [128, 64], dt.bfloat16, kind="ExternalInput")
    p3t = nc.dram_tensor("p3t", [64, 10], dt.bfloat16, kind="ExternalInput")
    cb2 = nc.dram_tensor("cb2", [64, 1], dt.float32, kind="ExternalInput")
    pb1 = nc.dram_tensor("pb1", [128, 1], dt.float32, kind="ExternalInput")
    pb2 = nc.dram_tensor("pb2", [64, 1], dt.float32, kind="ExternalInput")
    pb3 = nc.dram_tensor("pb3", [10, 1], dt.float32, kind="ExternalInput")
    out = nc.dram_tensor("out", [10, B_CORE], dt.float32, kind="ExternalOutput")
    debug = bool(_CACHE.get("debug"))
    if debug:
        dbg_et = nc.dram_tensor("dbg_et", [96, CH, 8, 16], dt.bfloat16, kind="ExternalOutput")
        dbg_ot = nc.dram_tensor("dbg_ot", [96, CH, 8, 16], dt.bfloat16, kind="ExternalOutput")
        dbg_p2f = nc.dram_tensor("dbg_p2f", [128, 28, B_CORE], dt.bfloat16, kind="ExternalOutput")
        dbg_feats = nc.dram_tensor("dbg_feats", [128, 2, 128], dt.bfloat16, kind="ExternalOutput")
        dbg_p1c = nc.dram_tensor("dbg_p1c", [128, CH, 7, 14], dt.bfloat16, kind="ExternalOutput")
        dbg_t3 = nc.dram_tensor("dbg_t3", [64, SUB, 7, 7], dt.bfloat16, kind="ExternalOutput")
        dbg_pe = nc.dram_tensor("dbg_pe", [64, 2, 448], dt.float32, kind="ExternalOutput")

    NEO = CH * 8 * 16  # flat elements per partition of an E/O tile

    with tile.TileContext(nc) as tc:
        with tc.tile_pool(name="singles", bufs=1) as singles:
            # conv1-critical loads on the sync HWDGE ring, first.
            w1_sb = singles.tile([19, 128], dt.bfloat16, tag="w1")
            nc.sync.dma_start(out=w1_sb, in_=w1[:, :], single_packet=True)
            # everything small on the scalar HWDGE ring.
            w2e_sb = singles.tile([96, 3, 64], dt.bfloat16, tag="w2e")
            nc.scalar.dma_start(out=w2e_sb, in_=w2e[:, :, :])
            w2o_sb = singles.tile([96, 3, 64], dt.bfloat16, tag="w2o")
            nc.scalar.dma_start(out=w2o_sb, in_=w2o[:, :, :])
            cb2_sb = singles.tile([64, 1], dt.float32, tag="cb2")
            nc.scalar.dma_start(out=cb2_sb, in_=cb2[:, :])
            fcb_sb = singles.tile([128, 2], dt.float32, tag="fcb")
            nc.scalar.dma_start(out=fcb_sb, in_=fcb[:, :])
            p1t_sb = singles.tile([8, 128], dt.bfloat16, tag="p1t")
            nc.scalar.dma_start(out=p1t_sb, in_=p1t[:, :])
            p2t_sb = singles.tile([128, 64], dt.bfloat16, tag="p2t")
            nc.scalar.dma_start(out=p2t_sb, in_=p2t[:, :])
            p3t_sb = singles.tile([64, 10], dt.bfloat16, tag="p3t")
            nc.scalar.dma_start(out=p3t_sb, in_=p3t[:, :])
            pb1_sb = singles.tile([128, 1], dt.float32, tag="pb1")
            nc.scalar.dma_start(out=pb1_sb, in_=pb1[:, :])
            pb2_sb = singles.tile([64, 1], dt.float32, tag="pb2")
            nc.scalar.dma_start(out=pb2_sb, in_=pb2[:, :])
            pb3_sb = singles.tile([10, 1], dt.float32, tag="pb3")
            nc.scalar.dma_start(out=pb3_sb, in_=pb3[:, :])

            # big tail weights: tiles now, DMAs emitted mid-conv-loop (sync ring)
            fcw_sb = singles.tile([128, 28, 2, 128], dt.bfloat16, tag="fcw")
            mrt_sb = singles.tile([128, 2, 2, 128], dt.bfloat16, tag="mrt")
            mit_sb = singles.tile([128, 2, 2, 128], dt.bfloat16, tag="mit")
            zext_sb = singles.tile([128, 2, 9], dt.bfloat16, tag="zext")

            # fc input [c + 64*(yp//4), (yp%4)*7+xp, s] -- sample-innermost so
            # the fc matmul rhs is contiguous (strided rhs slows the PE ~4x)
            p2full = singles.tile([128, 28, B_CORE], dt.bfloat16, tag="p2full")
            # j=21..27 of the upper half is never written; fc reads it with
            # zero weights, so it must at least be finite.
            nc.gpsimd.memset(p2full[64:128, 21:28, :], 0.0)

            ones18 = singles.tile([1, 8], dt.bfloat16, tag="ones18")
            nc.gpsimd.memset(ones18, 1.0)

            # per-chunk conv2 input tiles (persistent; only pads need zeroing,
            # emitted per-chunk inside the loop to keep engine queues clear)
            Et = [singles.tile([96, CH, 8, 16], dt.bfloat16, tag=f"Et{ci}",
                               name=f"Et{ci}") for ci in range(NCH)]
            Ot = [singles.tile([96, CH, 8, 16], dt.bfloat16, tag=f"Ot{ci}",
                               name=f"Ot{ci}") for ci in range(NCH)]

            with tc.tile_pool(name="ximp", bufs=8) as ximpool, \
                 tc.tile_pool(name="oddp", bufs=6) as oddpool, \
                 tc.tile_pool(name="p1cp", bufs=4) as p1cpool, \
                 tc.tile_pool(name="rcp", bufs=4) as rcpool, \
                 tc.tile_pool(name="t3p", bufs=6) as t3pool, \
                 tc.tile_pool(name="ps1", bufs=2, space="PSUM") as psum1, \
                 tc.tile_pool(name="ps2", bufs=2, space="PSUM") as psum2:
                p1cs = {}

                def emit_pads(ci):
                    # zero only the pad regions of Et/Ot (rest is overwritten)
                    ef = Et[ci].rearrange("p a b c -> p (a b c)")
                    of = Ot[ci].rearrange("p a b c -> p (a b c)")
                    nc.gpsimd.memset(Et[ci][0:32, :, :, 14:16], 0.0)
                    nc.gpsimd.memset(Et[ci][0:32, :, 7:8, 0:14], 0.0)
                    nc.gpsimd.memset(ef[32:64, 0:1], 0.0)
                    nc.gpsimd.memset(ef[64:96, NEO - 1:NEO], 0.0)
                    nc.gpsimd.memset(ef[64:96, NEO // 2 - 1:NEO // 2], 0.0)
                    nc.gpsimd.memset(Ot[ci][32:64, :, :, 14:16], 0.0)
                    nc.gpsimd.memset(Ot[ci][32:64, :, 0:1, 0:14], 0.0)
                    nc.gpsimd.memset(of[0:32, 0:1], 0.0)
                    nc.gpsimd.memset(of[64:96, NEO - 1:NEO], 0.0)
                    nc.gpsimd.memset(of[64:96, NEO // 2 - 1:NEO // 2], 0.0)

                def emit_conv1(ci, tt0, tt1):
                    xim_sb = xim_sbs[ci]
                    if tt0 == 0:
                        p1c = p1cpool.tile([128, CH, 7, 14], dt.bfloat16, tag="p1c")
                        p1cs[ci] = p1c
                    else:
                        p1c = p1cs[ci]
                    for tt in range(tt0, tt1):
                        c1p = psum1.tile([128, SUB // 2, 512], dt.float32, tag="c1p")
                        for sh in range(SUB // 2):
                            s0 = tt * SUB + sh * 2
                            if ci == 0 and tt < 2:
                                xsrc = xim0a[:, s0:s0 + 2]
                            else:
                                xsrc = xim_sb[:, s0:s0 + 2]
                            nc.tensor.matmul(
                                c1p[:, sh, 0:392].rearrange(
                                    "p (s q x) -> p s q x", s=2, q=7, x=28),
                                w1_sb, xsrc, start=True, stop=True)
                        c1v = c1p[:, :, 0:392].rearrange(
                            "p h (s q xp two) -> p h s q xp two", s=2, q=7, xp=14, two=2)
                        odd1 = oddpool.tile([128, SUB, 7, 14], dt.bfloat16, tag="odd1")
                        o1v = odd1.rearrange("p (h s) q xp -> p h s q xp", h=SUB // 2)
                        nc.scalar.activation(o1v, c1v[:, :, :, :, :, 1], AF.Copy)
                        nc.vector.scalar_tensor_tensor(
                            p1c[:, tt * SUB:(tt + 1) * SUB].rearrange(
                                "p (h s) q xp -> p h s q xp", h=SUB // 2),
                            c1v[:, :, :, :, :, 0], 0.0, o1v, ALU.max, ALU.max)

                def emit_pooly(ci):
                    p1c = p1cs[ci]
                    rc = rcpool.tile([64, CH, 7, 14], dt.bfloat16, tag="rc")
                    nc.vector.tensor_copy(out=rc, in_=p1c[64:128])
                    # E rows 0..6 = even pooled rows; O rows 1..7 = odd pooled rows
                    nc.vector.tensor_tensor(
                        Et[ci][0:32, :, 0:7, 0:14], p1c[0:32], rc[0:32], ALU.max)
                    nc.vector.tensor_tensor(
                        Ot[ci][32:64, :, 1:8, 0:14], p1c[32:64], rc[32:64], ALU.max)
                    ef = Et[ci].rearrange("p a b c -> p (a b c)")
                    of = Ot[ci].rearrange("p a b c -> p (a b c)")
                    # +-1 column shifts as single contiguous byte-shifted copies;
                    # sync ring (xims are pre-issued, so nothing queues behind these)
                    nc.sync.dma_start(out=ef[32:64, 1:NEO], in_=ef[0:32, 0:NEO - 1])
                    nc.sync.dma_start(out=ef[64:96, 0:NEO - 1], in_=ef[0:32, 1:NEO])
                    nc.sync.dma_start(out=of[0:32, 1:NEO], in_=of[32:64, 0:NEO - 1])
                    nc.sync.dma_start(out=of[64:96, 0:NEO - 1], in_=of[32:64, 1:NEO])

                def emit_conv2(ci):
                    for tt in range(SPC):
                        s0 = tt * SUB
                        ps = psum2.tile([64, 2, 512], dt.float32, tag="c2p")
                        pe = ps[:, 0, 0:448].rearrange(
                            "p (s y x) -> p s y x", s=SUB, y=7, x=16)[:, :, :, 0:14]
                        po = ps[:, 1, 0:448].rearrange(
                            "p (s y x) -> p s y x", s=SUB, y=7, x=16)[:, :, :, 0:14]
                        Ev = Et[ci][:, s0:s0 + SUB]
                        Ov = Ot[ci][:, s0:s0 + SUB]
                        # even out rows: W[0]*O[yy-1] + W[1]*E[yy] + W[2]*O[yy]
                        nc.tensor.matmul(pe, w2o_sb[:, 0, :], Ov[:, :, 0:7, 0:14],
                                         start=True, stop=False)
                        nc.tensor.matmul(pe, w2e_sb[:, 1, :], Ev[:, :, 0:7, 0:14],
                                         start=False, stop=False)
                        nc.tensor.matmul(pe, w2o_sb[:, 2, :], Ov[:, :, 1:8, 0:14],
                                         start=False, stop=True)
                        # odd out rows: W[0]*E[yy] + W[1]*O[yy] + W[2]*E[yy+1]
                        nc.tensor.matmul(po, w2e_sb[:, 0, :], Ev[:, :, 0:7, 0:14],
                                         start=True, stop=False)
                        nc.tensor.matmul(po, w2o_sb[:, 1, :], Ov[:, :, 1:8, 0:14],
                                         start=False, stop=False)
                        nc.tensor.matmul(po, w2e_sb[:, 2, :], Ev[:, :, 1:8, 0:14],
                                         start=False, stop=True)
                        # fused maxpool 2x2: max over (y-parity, x-pair); junk
                        # col pair (14,15) excluded from the input AP
                        t3 = t3pool.tile([64, SUB, 7, 7], dt.bfloat16, tag="t3")
                        rin = ps[:, :, 0:448].rearrange(
                            "p par (s y xh two) -> p s y xh par two",
                            s=SUB, y=7, xh=8, two=2)[:, :, :, 0:7]
                        nc.vector.tensor_reduce(t3, rin, axis=AX.XY, op=ALU.max)
                        if debug and ci == 0 and tt == 0:
                            nc.sync.dma_start(out=dbg_t3[:, :, :, :], in_=t3)
                            pecp = t3pool.tile([64, 2, 448], dt.float32, tag="pecp")
                            nc.scalar.activation(pecp, ps[:, :, 0:448], AF.Copy)
                            nc.sync.dma_start(out=dbg_pe[:, :, :], in_=pecp)
                        # relu + conv2 bias, scatter to fc layout
                        gs = ci * CH + s0
                        # iterate (ypm, xp, s) so dst writes are contiguous
                        # 4-sample runs instead of stride-128 singles
                        dst0 = p2full[0:64].rearrange(
                            "p (ypm xp) s -> p ypm xp s", ypm=4)[:, :, :, gs:gs + SUB]
                        dst1 = p2full[64:128, 0:21, :].rearrange(
                            "p (ypm xp) s -> p ypm xp s", ypm=3)[:, :, :, gs:gs + SUB]
                        if ci == NCH - 1 and tt == SPC - 1:
                            # final subtile: emit on vector right after its own
                            # reduce so fc isn't gated on a scalar-queue hop
                            nc.vector.tensor_scalar(
                                dst0, t3[:, :, 0:4, :].rearrange("p s y x -> p y x s"),
                                cb2_sb[:, 0:1], 0.0, ALU.add, ALU.max)
                            nc.vector.tensor_scalar(
                                dst1, t3[:, :, 4:7, :].rearrange("p s y x -> p y x s"),
                                cb2_sb[:, 0:1], 0.0, ALU.add, ALU.max)
                        else:
                            nc.scalar.activation(
                                dst0, t3[:, :, 0:4, :].rearrange("p s y x -> p y x s"),
                                AF.Relu, bias=cb2_sb[:, 0:1])
                            nc.scalar.activation(
                                dst1, t3[:, :, 4:7, :].rearrange("p s y x -> p y x s"),
                                AF.Relu, bias=cb2_sb[:, 0:1])

                LAG = 4
                xim0a = ximpool.tile([19, 2 * SUB, 7, 28], dt.bfloat16, tag="xim0a")
                nc.sync.dma_start(out=xim0a, in_=xim[:, 0:2 * SUB, :, :],
                                  single_packet=True)
                xim_sbs = {}
                for ci in range(NCH):
                    xim_sbs[ci] = ximpool.tile([19, CH, 7, 28], dt.bfloat16,
                                               tag="xim_sb", name=f"xim{ci}")
                    nc.sync.dma_start(out=xim_sbs[ci],
                                      in_=xim[:, ci * CH:(ci + 1) * CH, :, :])
                nc.sync.dma_start(out=fcw_sb, in_=fcw[:, :, :, :])
                nc.sync.dma_start(out=mrt_sb, in_=mrt[:, :, :, :])
                nc.sync.dma_start(out=mit_sb, in_=mit[:, :, :, :])
                nc.sync.dma_start(out=zext_sb, in_=zext[:, :, :])
                emit_pads(0)
                emit_pads(1)
                for ci in range(NCH):
                    emit_conv1(ci, 0, SPC)
                    if ci + 2 < NCH:
                        emit_pads(ci + 2)
                    emit_pooly(ci)
                    if ci >= LAG:
                        emit_conv2(ci - LAG)
                for ci in range(NCH - LAG, NCH):
                    emit_conv2(ci)
                if debug:
                    nc.sync.dma_start(out=dbg_et[:, :, :, :], in_=Et[0])
                    nc.sync.dma_start(out=dbg_ot[:, :, :, :], in_=Ot[0])

            # ---------------- dense tail ----------------
            with tc.tile_pool(name="tail", bufs=1) as tail, \
                 tc.tile_pool(name="psumT", bufs=1, space="PSUM") as psumT:
                fp = psumT.tile([128, 2, 128], dt.float32, tag="fp")
                feats = tail.tile([128, 2, 128], dt.bfloat16, tag="feats")
                for mt in range(2):
                    for j in range(28):
                        nc.tensor.matmul(
                            fp[:, mt], fcw_sb[:, j, mt, :], p2full[:, j, :],
                            start=(j == 0), stop=(j == 27))
                    # tanh(mt) overlaps the next mt's matmuls on scalar
                    nc.scalar.activation(feats[:, mt], fp[:, mt], AF.Tanh,
                                         bias=fcb_sb[:, mt:mt + 1])

                sq = psumT.tile([128, 4, 128], dt.float32, tag="sq")
                srp = sq[:, 0:2]
                sip = sq[:, 2:4]
                for mt in range(2):
                    for kb in range(2):
                        nc.tensor.matmul(srp[:, mt], mrt_sb[:, kb, mt, :], feats[:, kb],
                                         start=(kb == 0), stop=(kb == 1))
                    for kb in range(2):
                        nc.tensor.matmul(sip[:, mt], mit_sb[:, kb, mt, :], feats[:, kb],
                                         start=(kb == 0), stop=(kb == 1))

                if debug:
                    nc.sync.dma_start(out=dbg_p2f[:, :, :], in_=p2full)
                    nc.sync.dma_start(out=dbg_feats[:, :, :], in_=feats)

                probs = tail.tile([128, 2, 128], dt.bfloat16, tag="probs")
                for mt in range(2):
                    t1 = tail.tile([128, 128], dt.float32, tag=f"sq_r{mt}")
                    nc.scalar.activation(t1, srp[:, mt], AF.Square)
                    t2s = tail.tile([128, 128], dt.float32, tag=f"sq_i{mt}")
                    nc.scalar.activation(t2s, sip[:, mt], AF.Square)
                    nc.vector.tensor_tensor(probs[:, mt], t1, t2s, ALU.add)

                qt = psumT.tile([8, 2, 128], dt.float32, tag="qt")
                qp = qt[:, 0]
                tp = qt[0:1, 1]
                for kb in range(2):
                    nc.tensor.matmul(qp, zext_sb[:, kb, 0:8], probs[:, kb],
                                     start=(kb == 0), stop=(kb == 1))
                for kb in range(2):
                    nc.tensor.matmul(tp, zext_sb[:, kb, 8:9], probs[:, kb],
                                     start=(kb == 0), stop=(kb == 1))

                recip = tail.tile([1, 128], dt.float32, tag="recip")
                nc.vector.reciprocal_approx_fast(recip, tp)
                recip_bf = tail.tile([1, 128], dt.bfloat16, tag="recip_bf")
                nc.vector.tensor_copy(out=recip_bf, in_=recip)
                bcp = psumT.tile([8, 128], dt.float32, tag="bcp")
                nc.tensor.matmul(bcp, ones18, recip_bf, start=True, stop=True)
                bc_sb = tail.tile([8, 128], dt.bfloat16, tag="bc_sb")
                nc.scalar.activation(bc_sb, bcp, AF.Copy)

                qn = tail.tile([8, 128], dt.bfloat16, tag="qn")
                nc.vector.tensor_tensor(qn, qp, bc_sb, ALU.mult)

                z1p = psumT.tile([128, 128], dt.float32, tag="z1p")
                nc.tensor.matmul(z1p, p1t_sb, qn, start=True, stop=True)
                z1 = tail.tile([128, 128], dt.bfloat16, tag="z1")
                nc.scalar.activation(z1, z1p, AF.Relu, bias=pb1_sb[:, 0:1])

                z2p = psumT.tile([64, 128], dt.float32, tag="z2p")
                nc.tensor.matmul(z2p, p2t_sb, z1, start=True, stop=True)
                z2 = tail.tile([64, 128], dt.bfloat16, tag="z2")
                nc.scalar.activation(z2, z2p, AF.Relu, bias=pb2_sb[:, 0:1])

                z3p = psumT.tile([10, 128], dt.float32, tag="z3p")
                nc.tensor.matmul(z3p, p3t_sb, z2, start=True, stop=True)
                osb = tail.tile([10, 128], dt.float32, tag="osb")
                nc.vector.tensor_scalar_add(osb, z3p, pb3_sb[:, 0:1])
                nc.sync.dma_start(out=out[:, :], in_=osb)

    nc.finalize()
    return nc


def _get_nc():
    if "nc" not in _CACHE:
        _CACHE["nc"] = _build_bass()
    return _CACHE["nc"]


def kernel(**inputs) -> np.ndarray:
    from concourse.bass_utils import run_bass_kernel_spmd

    in_maps = _host_prep(inputs)
    nc = _get_nc()
    res = run_bass_kernel_spmd(nc, in_maps, core_ids=list(range(NCORES)),
                               trace=bool(_CACHE.get("trace")))
    _CACHE["last_result"] = res
    outs = [r["out"].T for r in res.results]  # each [128, 10]
    return np.ascontiguousarray(np.concatenate(outs, axis=0), dtype=np.float32)
